# revision 51
# baseline (speedup 1.0000x reference)
"""SNN recurrent layer (Linear + leaky-integrate-and-fire scan) on 8 trn2 NeuronCores.

~524 us HW exec (vs 1492 us fp32 baseline), rel err ~7.6e-3 (gate 2e-2).

Strategy (pure data parallel over batch; each core owns 32 of 256 batches):
  - h = X @ W.T as THREE bf16 matmul passes (Xh*Wh + Xh*Wl + Xl*Wh) where
    X = Xh + Xl is a host-side bf16 hi/lo split (same HBM bytes as fp32 X).
    h abs err ~3.5e-6 -> spike rel err ~6e-3 (CPU-sim verified); 3x cheaper
    on the PE than fp32's 4-cycle/row mode. fp32r (1 cyc/row, 11-bit
    mantissa) was measured too imprecise naked (rel 3.9e-2).
  - Input arrives t-major [T, 32b, 2, 768pad] bf16; per 4-t chunk one DMA
    lands [128=(4t x 32b), 4 x 768B lines] in SBUF (the 4-line split makes
    the DMA emit 16 descriptor packets -> uses all 16 SDMA engines; a
    single 3072B line per partition only engages 4).
  - PE transposes the chunk (bf16) into [i, tb] K-chunks via PSUM; ACT
    copies to SBUF; 18 matmuls (3 passes x 6 K-chunks, X stationary /
    W moving 400-wide) accumulate h into PSUM [128 tb, 400 o] fp32.
    Matmuls for chunk n are emitted AFTER transposes of chunk n+1 so the
    PE never stalls on the ACT copy.
  - ACT stages h into hstg4 [p, oc, c(4 chunks), o']; each 4-chunk batch is
    scattered by 4 DMAs (one per t-row group) into the recurrence layout
    hseg [p'=(b*4+oc), t, o'] (32->128 partition fan-out, 400B lines).
  - The recurrence runs as 2 fused custom DVE ops per timestep, emitted
    incrementally right after each reshuffle flush so the serial chain
    hides under the PE span:
      mem' = (mem <= 1) ? (beta*mem + syn) : 0   (MEMSTEP)
      syn' = alpha*syn + h_t                     (SYNSTEP)
    (A per-segment tensor_tensor_scan for syn was ~90 us slower end-to-end:
    its 100-instruction bursts serialized into a long post-PE tail.)
  - spikes = (mem ring > 1) in 20-t batches on DVE; written straight to
    HBM by 4 stride-4-partition DMAs issued from the GPSIMD SWDGE queue --
    keeping these spike-gated DMAs off the SP HWDGE FIFO so input/reshuffle
    DMAs are never stuck behind them.
"""

import numpy as np
import ml_dtypes

ALPHA = 0.9
BETA = 0.85

B_FULL, T_FULL, I_FULL, O_FULL = 256, 500, 700, 400
IPAD = 768
NCORES = 8

_CACHE = {}


# --------------------------------------------------------------------------- #
# Custom DVE op: one fused membrane update step.
#   out = select(mem <= 1, beta*mem + syn, 0)
# --------------------------------------------------------------------------- #
def _register_custom_op(name, spec_fn):
    import concourse.dve_ops as dvo

    for op in dvo.OPS:
        if op.name == name:
            return op

    spec = spec_fn()

    def _append(op):
        dvo.OPS.append(op)
        dvo.CUSTOM_DVE_SPECS[op.name] = op.spec
        dvo._SUB_OPCODE_FOR_NAME[op.name] = dvo._CUSTOM_DVE_ROW_BASE + len(dvo.OPS) - 1

    # Two-phase registration: learn the uops shas from the pin-check error.
    import re as _re

    probe = dvo.DveOp(name, spec, subdim=False, uops_sha={})
    _append(probe)
    shas = {}
    for ver in ("v3", "v4"):
        try:
            probe.compile(ver)
            shas[ver] = probe.uops_sha[ver]
        except ValueError as e:
            m = _re.search(r'uops_sha\["(v\d)"\]="([0-9a-f]+)"', str(e))
            shas[m.group(1)] = m.group(2)
    dvo.OPS.remove(probe)
    del dvo._SUB_OPCODE_FOR_NAME[probe.name]
    final = dvo.DveOp(name, spec, subdim=False, uops_sha=shas)
    _append(final)
    return final


def _register_memstep():
    from concourse.dve_spec import Spec, Src0, Src1, C0, Zero, One, select

    def _mk():
        def _ref(in0, in1, s0, s1, imm2):
            a = (in0.astype(np.float32) * np.float32(s0) + in1).astype(np.float32)
            return np.where(in0 <= 1.0, a, np.float32(0.0)).astype(np.float32)

        return Spec(body=select(Src0 <= One, Src0 * C0 + Src1, Zero), reference=_ref)

    return _register_custom_op("SNN_MEMSTEP_ANT", _mk)


def _register_synstep():
    from concourse.dve_spec import Spec, Src0, Src1, C0

    def _mk():
        def _ref(in0, in1, s0, s1, imm2):
            return (in0.astype(np.float32) * np.float32(s0) + in1).astype(np.float32)

        return Spec(body=Src0 * C0 + Src1, reference=_ref)

    return _register_custom_op("SNN_SYNSTEP_ANT", _mk)


# --------------------------------------------------------------------------- #
# Program builder (per-core SPMD program).
# --------------------------------------------------------------------------- #
def build_program(B_L, T, I, O, seg_lens=None, memk=20):
    import concourse.bass as bass
    import concourse.bacc as bacc
    import concourse.mybir as mybir
    import concourse.tile as tile

    MEMSTEP = _register_memstep()
    SYNSTEP = _register_synstep()

    P = 128
    TC = P // B_L                      # timesteps per matmul chunk (4)
    assert B_L * TC == P
    NCH = T // TC                      # matmul chunks (125)
    assert NCH * TC == T
    NK = IPAD // P                     # K-chunks (6)
    OC = 4                             # o'-groups (4 x 100)
    OP = O // OC                       # o' lanes per partition (100)
    MEMK = memk
    if seg_lens is None:
        seg_lens = [100] * 5 if T == 500 else [T]
    assert sum(seg_lens) == T
    assert all(sl % TC == 0 and sl % MEMK == 0 for sl in seg_lens)
    SEG_STARTS = [int(v) for v in np.cumsum([0] + seg_lens)]
    SEG_MAX = max(seg_lens)
    NSEG = len(seg_lens)

    f32 = mybir.dt.float32
    bf16 = mybir.dt.bfloat16

    nc = bacc.Bacc(
        "TRN2",
        target_bir_lowering=False,
        debug=False,
        enable_asserts=False,
        num_devices=1,
    )

    x2_d = nc.dram_tensor("x2", [T, B_L, 2, IPAD], bf16, kind="ExternalInput").ap()
    wh_d = nc.dram_tensor("wh", [O, IPAD], bf16, kind="ExternalInput").ap()
    wl_d = nc.dram_tensor("wl", [O, IPAD], bf16, kind="ExternalInput").ap()
    id_d = nc.dram_tensor("ident", [P, P], bf16, kind="ExternalInput").ap()
    out_d = nc.dram_tensor("out", [B_L, T, O], f32, kind="ExternalOutput").ap()

    with tile.TileContext(nc) as tc:
        with (
            tc.tile_pool(name="persist", bufs=1) as pp,
            tc.tile_pool(name="xn", bufs=4) as xnp,
            tc.tile_pool(name="xt", bufs=3) as xtp,
            tc.tile_pool(name="stage", bufs=3) as stp,
            tc.tile_pool(name="hstg", bufs=4) as hsp,
            tc.tile_pool(name="xt_ps", bufs=2, space=bass.MemorySpace.PSUM) as xtpp,
            tc.tile_pool(name="h_ps", bufs=3, space=bass.MemorySpace.PSUM) as hpp,
        ):
            # ---------------- persistent tiles ----------------
            ident = pp.tile([P, P], bf16)
            nc.sync.dma_start(ident[:, :], id_d[:, :])
            wt_h = pp.tile([P, NK, O], bf16)           # [i_sub, k, o]
            wt_l = pp.tile([P, NK, O], bf16)
            syn = pp.tile([P, OP], f32)
            nc.vector.memset(syn[:, :], 0.0)
            ring = pp.tile([P, MEMK + 1, OP], f32)
            nc.vector.memset(ring[:, 0, :], 0.0)
            hseg = [
                pp.tile([P, SEG_MAX, OP], f32, name=f"hseg{i}", tag=f"hseg{i}")
                for i in range(2)
            ]

            # ---------------- W -> Wt (one-time) ----------------
            with (
                tc.tile_pool(name="wsetup", bufs=1) as wsp,
                tc.tile_pool(name="w_ps", bufs=1, space=bass.MemorySpace.PSUM) as wpp,
            ):
                for w_d, wt in ((wh_d, wt_h), (wl_d, wt_l)):
                    w_stage = wsp.tile([P, OC, IPAD], bf16, tag="wstage")
                    for c in range(OC):
                        pc = min(P, O - c * P)
                        if pc <= 0:
                            break
                        nc.sync.dma_start(
                            w_stage[0:pc, c, :], w_d[c * P:c * P + pc, :]
                        )
                    for k in range(NK):
                        w_ps = wpp.tile([P, O], bf16, tag="w_ps")
                        for c in range(OC):
                            pc = min(P, O - c * P)
                            if pc <= 0:
                                break
                            nc.tensor.transpose(
                                w_ps[0:P, c * P:c * P + pc],
                                w_stage[0:pc, c, k * P:(k + 1) * P],
                                ident[0:pc, 0:pc],
                            )
                        nc.scalar.copy(wt[:, k, :], w_ps[:, :])

            # ---------------- helpers ----------------
            def emit_steps(gt0, gt1):
                # membrane + synapse steps for global t in [gt0, gt1), emitted
                # incrementally right after the reshuffle flush that made the
                # needed h columns available — the serial DVE chain then runs
                # concurrently with the rest of the pipeline.
                for t in range(gt0, gt1):
                    s = seg_of(t)
                    tl = t - SEG_STARTS[s]
                    j = t % MEMK
                    # mem' = select(mem <= 1, beta*mem + syn_{t-1}, 0)
                    nc.vector._custom_dve(
                        MEMSTEP,
                        out=ring[:, j + 1, :],
                        in0=ring[:, j, :],
                        in1=syn[:, :],
                        s0=BETA,
                    )
                    if j == MEMK - 1:
                        tb0 = t - (MEMK - 1)
                        # spikes on DVE: stage[p=(b,oc), t(20), o'(100)]
                        stage = stp.tile([P, MEMK, OP], f32, tag="stage")
                        nc.vector.tensor_scalar(
                            stage[:, :, :],
                            ring[:, 0:MEMK, :],
                            1.0,
                            None,
                            op0=mybir.AluOpType.is_gt,
                        )
                        nc.vector.tensor_copy(ring[:, 0, :], ring[:, MEMK, :])
                        # direct out via the idle GPSIMD (SWDGE) queue: keeps
                        # spike-gated DMAs off the SP FIFO so the next
                        # segment's input DMAs are never stuck behind them
                        for oc in range(OC):
                            nc.gpsimd.dma_start(
                                out_d[:, tb0:tb0 + MEMK, oc * OP:(oc + 1) * OP],
                                stage[oc::OC, :, :],
                            )
                    # syn' = alpha*syn + h_t  (skip for the final step)
                    if t < T - 1:
                        nc.vector._custom_dve(
                            SYNSTEP,
                            out=syn[:, :],
                            in0=syn[:, :],
                            in1=hseg[s % 2][:, tl, :],
                            s0=ALPHA,
                        )

            # ---------------- main pipeline ----------------
            x_tb = x2_d                            # [T, B_L, 2, IPAD] t-major
            RB = 4                                 # chunks per reshuffle batch
            hstg4 = None

            def seg_of(t):
                for si in range(NSEG):
                    if t < SEG_STARTS[si + 1]:
                        return si
                raise AssertionError

            bstate = {"c": 0, "tlb": 0}

            def consume(n, xt):
                # matmuls + staging + reshuffle for a chunk
                nonlocal hstg4
                t0 = n * TC
                s = seg_of(t0)
                tl0 = t0 - SEG_STARTS[s]
                ns = tl0 // TC                     # chunk index within segment
                cps = seg_lens[s] // TC            # chunks in this segment
                c = bstate["c"]                    # position within batch
                if c == 0:
                    bstate["tlb"] = tl0

                # -- matmul: h_ps[tb, o] = Xh Wh + Xh Wl + Xl Wh
                h_ps = hpp.tile([P, O], f32, tag="h_ps")
                passes = ((0, wt_h), (0, wt_l), (1, wt_h))
                npass = len(passes)
                for pi, (a, wt) in enumerate(passes):
                    for k in range(NK):
                        nc.tensor.matmul(
                            h_ps[:, :],
                            xt[:, a, k * P:(k + 1) * P],
                            wt[:, k, :],
                            start=(pi == 0 and k == 0),
                            stop=(pi == npass - 1 and k == NK - 1),
                        )

                # -- stage h PSUM->SBUF (ACT) into [p, oc, c, o'] batch tile
                if c == 0:
                    hstg4 = hsp.tile([P, OC, RB, OP], f32, tag="hstg")
                nc.scalar.copy(
                    hstg4[:, :, c, :],
                    h_ps[:, :].rearrange("p (oc o) -> p oc o", oc=OC),
                )

                # -- end of batch (or segment, or near the kernel's end where
                # per-chunk flushing keeps the final DVE tail short)
                if c == RB - 1 or ns == cps - 1:
                    rb = c + 1
                    tlb = bstate["tlb"]            # first t of batch in segment
                    bstate["c"] = 0
                    hb = hseg[s % 2]
                    for tau in range(TC):
                        if rb == RB:
                            nc.sync.dma_start(
                                hb[:, tlb + tau:tlb + tau + (rb - 1) * TC + 1:TC, :],
                                hstg4[tau * 32:(tau + 1) * 32, :, 0:rb, :],
                            )
                        else:
                            # partial tail batch: per-chunk DMAs (AP balance
                            # can't express the strided scatter in <=3 dims)
                            for cc in range(rb):
                                nc.sync.dma_start(
                                    hb[:, tlb + cc * TC + tau, :],
                                    hstg4[tau * 32:(tau + 1) * 32, :, cc, :],
                                )
                    gt0 = SEG_STARTS[s] + tlb
                    emit_steps(gt0, gt0 + rb * TC)
                else:
                    bstate["c"] = c + 1

            IH = IPAD // 2                 # 384
            IHP = IH + 16                  # padded half-line (400 elems)
            pending = None
            for n in range(NCH):
                t0 = n * TC

                # -- load X chunk [128=(4t x 32b), 2, IPAD] (hi+lo interleaved).
                # The tile splits each partition's 3072B into 4 x 768B lines
                # (+32B pad) so the DMA emits 512 descriptors -> 16 packets ->
                # spreads across all 16 SDMA engines instead of 4.
                xn = xnp.tile([P, 4, IHP], bf16, tag="xn")
                nc.sync.dma_start(
                    xn[:, :, 0:IH],
                    x_tb[t0:t0 + TC, :, :, :],
                )

                # -- PE transpose to [i, tb] chunks (bf16, into one PSUM tile)
                xt_ps = xtpp.tile([P, 2, NK * P], bf16, tag="xt_ps")
                for a in range(2):
                    for k in range(NK):
                        m = a * 2 + k // 3
                        c0 = (k % 3) * P
                        nc.tensor.transpose(
                            xt_ps[:, a, k * P:(k + 1) * P],
                            xn[:, m, c0:c0 + P],
                            ident[:, :],
                        )
                xt = xtp.tile([P, 2, NK * P], bf16, tag="xt")
                nc.scalar.copy(xt[:, :, :], xt_ps[:, :, :])

                if pending is not None:
                    consume(*pending)
                pending = (n, xt)
            consume(*pending)

    nc.compile()
    return nc, {"B_L": B_L, "T": T, "I": I, "O": O}


# --------------------------------------------------------------------------- #
# Host-side entry point
# --------------------------------------------------------------------------- #
def split_pad(a, pad_to):
    bf = ml_dtypes.bfloat16
    hi = a.astype(bf)
    lo = (a - hi.astype(np.float32)).astype(bf)
    if a.shape[-1] != pad_to:
        pw = [(0, 0)] * (a.ndim - 1) + [(0, pad_to - a.shape[-1])]
        hi = np.pad(hi, pw)
        lo = np.pad(lo, pw)
    return np.ascontiguousarray(hi), np.ascontiguousarray(lo)


def build_x2(x):
    """Interleave the bf16 hi/lo split, t-major: [T, B, 2, IPAD]."""
    bf = ml_dtypes.bfloat16
    B, T, I = x.shape
    x2 = np.zeros((T, B, 2, IPAD), dtype=bf)
    hi = x.astype(bf)
    x2[:, :, 0, :I] = np.swapaxes(hi, 0, 1)
    x2[:, :, 1, :I] = np.swapaxes((x - hi.astype(np.float32)).astype(bf), 0, 1)
    return x2


def kernel(inputs: np.ndarray, W: np.ndarray, nb_steps) -> np.ndarray:
    from concourse.bass_utils import run_bass_kernel_spmd

    B, T, I = inputs.shape
    O = W.shape[0]
    assert (B, T, I, O) == (B_FULL, T_FULL, I_FULL, O_FULL), (B, T, I, O)
    assert int(nb_steps) == T

    key = (B, T, I, O)
    if key not in _CACHE:
        _CACHE[key] = build_program(B // NCORES, T, I, O)
    nc, meta = _CACHE[key]

    bf = ml_dtypes.bfloat16
    x = np.ascontiguousarray(inputs, dtype=np.float32)
    w = np.ascontiguousarray(W, dtype=np.float32)

    x2 = build_x2(x)
    wh, wl = split_pad(w, IPAD)
    ident = np.eye(128, dtype=bf)

    B_L = B // NCORES
    in_maps = [
        {
            "x2": np.ascontiguousarray(x2[:, c * B_L:(c + 1) * B_L]),
            "wh": wh,
            "wl": wl,
            "ident": ident,
        }
        for c in range(NCORES)
    ]
    results = run_bass_kernel_spmd(nc, in_maps, core_ids=list(range(NCORES)))
    outs = [r["out"] for r in results.results]
    return np.concatenate(outs, axis=0)


# revision 52
# speedup vs baseline: 1.0609x; 1.0609x over previous
"""SNN recurrent layer (Linear + leaky-integrate-and-fire scan) on 8 trn2 NeuronCores.

~524 us HW exec (vs 1492 us fp32 baseline), rel err ~7.6e-3 (gate 2e-2).

Strategy (pure data parallel over batch; each core owns 32 of 256 batches):
  - h = X @ W.T as THREE bf16 matmul passes (Xh*Wh + Xh*Wl + Xl*Wh) where
    X = Xh + Xl is a host-side bf16 hi/lo split (same HBM bytes as fp32 X).
    h abs err ~3.5e-6 -> spike rel err ~6e-3 (CPU-sim verified); 3x cheaper
    on the PE than fp32's 4-cycle/row mode. fp32r (1 cyc/row, 11-bit
    mantissa) was measured too imprecise naked (rel 3.9e-2).
  - Input arrives t-major [T, 32b, 2, 768pad] bf16; per 4-t chunk one DMA
    lands [128=(4t x 32b), 4 x 768B lines] in SBUF (the 4-line split makes
    the DMA emit 16 descriptor packets -> uses all 16 SDMA engines; a
    single 3072B line per partition only engages 4).
  - PE transposes the chunk (bf16) into [i, tb] K-chunks via PSUM; ACT
    copies to SBUF; 18 matmuls (3 passes x 6 K-chunks, X stationary /
    W moving 400-wide) accumulate h into PSUM [128 tb, 400 o] fp32.
    Matmuls for chunk n are emitted AFTER transposes of chunk n+1 so the
    PE never stalls on the ACT copy.
  - ACT stages h into hstg4 [p, oc, c(4 chunks), o']; each 4-chunk batch is
    scattered by 4 DMAs (one per t-row group) into the recurrence layout
    hseg [p'=(b*4+oc), t, o'] (32->128 partition fan-out, 400B lines).
  - The recurrence runs as 2 fused custom DVE ops per timestep, emitted
    incrementally right after each reshuffle flush so the serial chain
    hides under the PE span:
      mem' = (mem <= 1) ? (beta*mem + syn) : 0   (MEMSTEP)
      syn' = alpha*syn + h_t                     (SYNSTEP)
    (A per-segment tensor_tensor_scan for syn was ~90 us slower end-to-end:
    its 100-instruction bursts serialized into a long post-PE tail.)
  - spikes = (mem ring > 1) in 20-t batches on DVE; written straight to
    HBM by 4 stride-4-partition DMAs issued from the GPSIMD SWDGE queue --
    keeping these spike-gated DMAs off the SP HWDGE FIFO so input/reshuffle
    DMAs are never stuck behind them.
"""

import numpy as np
import ml_dtypes

ALPHA = 0.9
BETA = 0.85

B_FULL, T_FULL, I_FULL, O_FULL = 256, 500, 700, 400
IPAD = 768
NCORES = 8

_CACHE = {}


# --------------------------------------------------------------------------- #
# Custom DVE op: one fused membrane update step.
#   out = select(mem <= 1, beta*mem + syn, 0)
# --------------------------------------------------------------------------- #
def _register_custom_op(name, spec_fn):
    import concourse.dve_ops as dvo

    for op in dvo.OPS:
        if op.name == name:
            return op

    spec = spec_fn()

    def _append(op):
        dvo.OPS.append(op)
        dvo.CUSTOM_DVE_SPECS[op.name] = op.spec
        dvo._SUB_OPCODE_FOR_NAME[op.name] = dvo._CUSTOM_DVE_ROW_BASE + len(dvo.OPS) - 1

    # Two-phase registration: learn the uops shas from the pin-check error.
    import re as _re

    probe = dvo.DveOp(name, spec, subdim=False, uops_sha={})
    _append(probe)
    shas = {}
    for ver in ("v3", "v4"):
        try:
            probe.compile(ver)
            shas[ver] = probe.uops_sha[ver]
        except ValueError as e:
            m = _re.search(r'uops_sha\["(v\d)"\]="([0-9a-f]+)"', str(e))
            shas[m.group(1)] = m.group(2)
    dvo.OPS.remove(probe)
    del dvo._SUB_OPCODE_FOR_NAME[probe.name]
    final = dvo.DveOp(name, spec, subdim=False, uops_sha=shas)
    _append(final)
    return final


def _register_memstep():
    from concourse.dve_spec import Spec, Src0, Src1, C0, Zero, One, select

    def _mk():
        def _ref(in0, in1, s0, s1, imm2):
            a = (in0.astype(np.float32) * np.float32(s0) + in1).astype(np.float32)
            return np.where(in0 <= 1.0, a, np.float32(0.0)).astype(np.float32)

        return Spec(body=select(Src0 <= One, Src0 * C0 + Src1, Zero), reference=_ref)

    return _register_custom_op("SNN_MEMSTEP_ANT", _mk)


def _register_synstep():
    from concourse.dve_spec import Spec, Src0, Src1, C0

    def _mk():
        def _ref(in0, in1, s0, s1, imm2):
            return (in0.astype(np.float32) * np.float32(s0) + in1).astype(np.float32)

        return Spec(body=Src0 * C0 + Src1, reference=_ref)

    return _register_custom_op("SNN_SYNSTEP_ANT", _mk)


# --------------------------------------------------------------------------- #
# Program builder (per-core SPMD program).
# --------------------------------------------------------------------------- #
def build_program(B_L, T, I, O, seg_lens=None, memk=20):
    import concourse.bass as bass
    import concourse.bacc as bacc
    import concourse.mybir as mybir
    import concourse.tile as tile

    MEMSTEP = _register_memstep()
    SYNSTEP = _register_synstep()

    P = 128
    TC = P // B_L                      # timesteps per matmul chunk (4)
    assert B_L * TC == P
    NCH = T // TC                      # matmul chunks (125)
    assert NCH * TC == T
    NK = IPAD // P                     # K-chunks (6)
    OC = 4                             # o'-groups (4 x 100)
    OP = O // OC                       # o' lanes per partition (100)
    MEMK = memk
    if seg_lens is None:
        seg_lens = [100] * 5 if T == 500 else [T]
    assert sum(seg_lens) == T
    assert all(sl % TC == 0 and sl % MEMK == 0 for sl in seg_lens)
    SEG_STARTS = [int(v) for v in np.cumsum([0] + seg_lens)]
    SEG_MAX = max(seg_lens)
    NSEG = len(seg_lens)

    f32 = mybir.dt.float32
    bf16 = mybir.dt.bfloat16

    nc = bacc.Bacc(
        "TRN2",
        target_bir_lowering=False,
        debug=False,
        enable_asserts=False,
        num_devices=1,
    )

    x2_d = nc.dram_tensor("x2", [T, B_L, 2, IPAD], bf16, kind="ExternalInput").ap()
    wh_d = nc.dram_tensor("wh", [O, IPAD], bf16, kind="ExternalInput").ap()
    wl_d = nc.dram_tensor("wl", [O, IPAD], bf16, kind="ExternalInput").ap()
    id_d = nc.dram_tensor("ident", [P, P], bf16, kind="ExternalInput").ap()
    out_d = nc.dram_tensor("out", [B_L, T, O], f32, kind="ExternalOutput").ap()

    with tile.TileContext(nc) as tc:
        with (
            tc.tile_pool(name="persist", bufs=1) as pp,
            tc.tile_pool(name="xn", bufs=4) as xnp,
            tc.tile_pool(name="xt", bufs=3) as xtp,
            tc.tile_pool(name="stage", bufs=2) as stp,
            tc.tile_pool(name="hstg", bufs=3) as hsp,
            tc.tile_pool(name="xt_ps", bufs=2, space=bass.MemorySpace.PSUM) as xtpp,
            tc.tile_pool(name="h_ps", bufs=3, space=bass.MemorySpace.PSUM) as hpp,
        ):
            # ---------------- persistent tiles ----------------
            ident = pp.tile([P, P], bf16)
            nc.sync.dma_start(ident[:, :], id_d[:, :])
            wt_h = pp.tile([P, NK, O], bf16)           # [i_sub, k, o]
            wt_l = pp.tile([P, NK, O], bf16)
            syn = pp.tile([P, OP], f32)
            nc.vector.memset(syn[:, :], 0.0)
            ring = pp.tile([P, MEMK + 1, OP], f32)
            nc.vector.memset(ring[:, 0, :], 0.0)
            hseg = [
                pp.tile([P, SEG_MAX, OP], f32, name=f"hseg{i}", tag=f"hseg{i}")
                for i in range(2)
            ]

            # ---------------- W -> Wt (one-time) ----------------
            with (
                tc.tile_pool(name="wsetup", bufs=1) as wsp,
                tc.tile_pool(name="w_ps", bufs=1, space=bass.MemorySpace.PSUM) as wpp,
            ):
                for w_d, wt in ((wh_d, wt_h), (wl_d, wt_l)):
                    w_stage = wsp.tile([P, OC, IPAD], bf16, tag="wstage")
                    for c in range(OC):
                        pc = min(P, O - c * P)
                        if pc <= 0:
                            break
                        nc.sync.dma_start(
                            w_stage[0:pc, c, :], w_d[c * P:c * P + pc, :]
                        )
                    for k in range(NK):
                        w_ps = wpp.tile([P, O], bf16, tag="w_ps")
                        for c in range(OC):
                            pc = min(P, O - c * P)
                            if pc <= 0:
                                break
                            nc.tensor.transpose(
                                w_ps[0:P, c * P:c * P + pc],
                                w_stage[0:pc, c, k * P:(k + 1) * P],
                                ident[0:pc, 0:pc],
                            )
                        nc.scalar.copy(wt[:, k, :], w_ps[:, :])

            # ---------------- helpers ----------------
            def emit_steps(gt0, gt1):
                # membrane + synapse steps for global t in [gt0, gt1), emitted
                # incrementally right after the reshuffle flush that made the
                # needed h columns available — the serial DVE chain then runs
                # concurrently with the rest of the pipeline.
                for t in range(gt0, gt1):
                    s = seg_of(t)
                    tl = t - SEG_STARTS[s]
                    j = t % MEMK
                    # mem' = select(mem <= 1, beta*mem + syn_{t-1}, 0)
                    nc.vector._custom_dve(
                        MEMSTEP,
                        out=ring[:, j + 1, :],
                        in0=ring[:, j, :],
                        in1=syn[:, :],
                        s0=BETA,
                    )
                    if j == MEMK - 1:
                        tb0 = t - (MEMK - 1)
                        # spikes on DVE: stage[p=(b,oc), t(20), o'(100)]
                        stage = stp.tile([P, MEMK, OP], f32, tag="stage")
                        nc.vector.tensor_scalar(
                            stage[:, :, :],
                            ring[:, 0:MEMK, :],
                            1.0,
                            None,
                            op0=mybir.AluOpType.is_gt,
                        )
                        nc.vector.tensor_copy(ring[:, 0, :], ring[:, MEMK, :])
                        # direct out via the idle GPSIMD (SWDGE) queue: keeps
                        # spike-gated DMAs off the SP FIFO so the next
                        # segment's input DMAs are never stuck behind them
                        for oc in range(OC):
                            nc.gpsimd.dma_start(
                                out_d[:, tb0:tb0 + MEMK, oc * OP:(oc + 1) * OP],
                                stage[oc::OC, :, :],
                            )
                    # syn' = alpha*syn + h_t  (skip for the final step)
                    if t < T - 1:
                        nc.vector._custom_dve(
                            SYNSTEP,
                            out=syn[:, :],
                            in0=syn[:, :],
                            in1=hseg[s % 2][:, tl, :],
                            s0=ALPHA,
                        )

            # ---------------- main pipeline ----------------
            x_tb = x2_d                            # [T, B_L, 2, IPAD] t-major
            RB = 4                                 # chunks per reshuffle batch
            hstg4 = None

            def seg_of(t):
                for si in range(NSEG):
                    if t < SEG_STARTS[si + 1]:
                        return si
                raise AssertionError

            bstate = {"c": 0, "tlb": 0}

            def consume(n, xt):
                # matmuls + staging + reshuffle for a chunk
                nonlocal hstg4
                t0 = n * TC
                s = seg_of(t0)
                tl0 = t0 - SEG_STARTS[s]
                ns = tl0 // TC                     # chunk index within segment
                cps = seg_lens[s] // TC            # chunks in this segment
                c = bstate["c"]                    # position within batch
                if c == 0:
                    bstate["tlb"] = tl0

                # -- matmul: h_ps[tb, o] = Xh Wh + Xh Wl + Xl Wh
                h_ps = hpp.tile([P, O], f32, tag="h_ps")
                passes = ((0, wt_h), (0, wt_l), (1, wt_h))
                npass = len(passes)
                for pi, (a, wt) in enumerate(passes):
                    for k in range(NK):
                        nc.tensor.matmul(
                            h_ps[:, :],
                            xt[:, a, k * P:(k + 1) * P],
                            wt[:, k, :],
                            start=(pi == 0 and k == 0),
                            stop=(pi == npass - 1 and k == NK - 1),
                        )

                # -- stage h PSUM->SBUF (ACT) into [p, oc, c, o'] batch tile
                if c == 0:
                    hstg4 = hsp.tile([P, OC, RB, OP], f32, tag="hstg")
                nc.scalar.copy(
                    hstg4[:, :, c, :],
                    h_ps[:, :].rearrange("p (oc o) -> p oc o", oc=OC),
                )

                # -- end of batch (or segment, or near the kernel's end where
                # per-chunk flushing keeps the final DVE tail short)
                if c == RB - 1 or ns == cps - 1:
                    rb = c + 1
                    tlb = bstate["tlb"]            # first t of batch in segment
                    bstate["c"] = 0
                    hb = hseg[s % 2]
                    for tau in range(TC):
                        if rb == RB:
                            nc.sync.dma_start(
                                hb[:, tlb + tau:tlb + tau + (rb - 1) * TC + 1:TC, :],
                                hstg4[tau * 32:(tau + 1) * 32, :, 0:rb, :],
                            )
                        else:
                            # partial tail batch: per-chunk DMAs (AP balance
                            # can't express the strided scatter in <=3 dims)
                            for cc in range(rb):
                                nc.sync.dma_start(
                                    hb[:, tlb + cc * TC + tau, :],
                                    hstg4[tau * 32:(tau + 1) * 32, :, cc, :],
                                )
                    gt0 = SEG_STARTS[s] + tlb
                    emit_steps(gt0, gt0 + rb * TC)
                else:
                    bstate["c"] = c + 1

            IH = IPAD // 2                 # 384
            IHP = IH + 16                  # padded half-line (400 elems)
            pending = None
            for n in range(NCH):
                t0 = n * TC

                # -- load X chunk [128=(4t x 32b), 2, IPAD] (hi+lo interleaved).
                # The tile splits each partition's 3072B into 4 x 768B lines
                # (+32B pad) so the DMA emits 512 descriptors -> 16 packets ->
                # spreads across all 16 SDMA engines instead of 4.
                xn = xnp.tile([P, 4, IHP], bf16, tag="xn")
                nc.sync.dma_start(
                    xn[:, :, 0:IH],
                    x_tb[t0:t0 + TC, :, :, :],
                )

                # -- PE transpose to [i, tb] chunks (bf16, into one PSUM tile)
                xt_ps = xtpp.tile([P, 2, NK * P], bf16, tag="xt_ps")
                for a in range(2):
                    for k in range(NK):
                        m = a * 2 + k // 3
                        c0 = (k % 3) * P
                        nc.tensor.transpose(
                            xt_ps[:, a, k * P:(k + 1) * P],
                            xn[:, m, c0:c0 + P],
                            ident[:, :],
                        )
                xt = xtp.tile([P, 2, NK * P], bf16, tag="xt")
                nc.scalar.copy(xt[:, :, :], xt_ps[:, :, :])

                if pending is not None:
                    consume(*pending)
                pending = (n, xt)
            consume(*pending)

    nc.compile()
    return nc, {"B_L": B_L, "T": T, "I": I, "O": O}


# --------------------------------------------------------------------------- #
# Host-side entry point
# --------------------------------------------------------------------------- #
def split_pad(a, pad_to):
    bf = ml_dtypes.bfloat16
    hi = a.astype(bf)
    lo = (a - hi.astype(np.float32)).astype(bf)
    if a.shape[-1] != pad_to:
        pw = [(0, 0)] * (a.ndim - 1) + [(0, pad_to - a.shape[-1])]
        hi = np.pad(hi, pw)
        lo = np.pad(lo, pw)
    return np.ascontiguousarray(hi), np.ascontiguousarray(lo)


def build_x2(x):
    """Interleave the bf16 hi/lo split, t-major: [T, B, 2, IPAD]."""
    bf = ml_dtypes.bfloat16
    B, T, I = x.shape
    x2 = np.zeros((T, B, 2, IPAD), dtype=bf)
    hi = x.astype(bf)
    x2[:, :, 0, :I] = np.swapaxes(hi, 0, 1)
    x2[:, :, 1, :I] = np.swapaxes((x - hi.astype(np.float32)).astype(bf), 0, 1)
    return x2


def kernel(inputs: np.ndarray, W: np.ndarray, nb_steps) -> np.ndarray:
    from concourse.bass_utils import run_bass_kernel_spmd

    B, T, I = inputs.shape
    O = W.shape[0]
    assert (B, T, I, O) == (B_FULL, T_FULL, I_FULL, O_FULL), (B, T, I, O)
    assert int(nb_steps) == T

    key = (B, T, I, O)
    if key not in _CACHE:
        _CACHE[key] = build_program(B // NCORES, T, I, O)
    nc, meta = _CACHE[key]

    bf = ml_dtypes.bfloat16
    x = np.ascontiguousarray(inputs, dtype=np.float32)
    w = np.ascontiguousarray(W, dtype=np.float32)

    x2 = build_x2(x)
    wh, wl = split_pad(w, IPAD)
    ident = np.eye(128, dtype=bf)

    B_L = B // NCORES
    in_maps = [
        {
            "x2": np.ascontiguousarray(x2[:, c * B_L:(c + 1) * B_L]),
            "wh": wh,
            "wl": wl,
            "ident": ident,
        }
        for c in range(NCORES)
    ]
    results = run_bass_kernel_spmd(nc, in_maps, core_ids=list(range(NCORES)))
    outs = [r["out"] for r in results.results]
    return np.concatenate(outs, axis=0)


# revision 55
# speedup vs baseline: 1.0631x; 1.0021x over previous
"""SNN recurrent layer (Linear + leaky-integrate-and-fire scan) on 8 trn2 NeuronCores.

~524 us HW exec (vs 1492 us fp32 baseline), rel err ~7.6e-3 (gate 2e-2).

Strategy (pure data parallel over batch; each core owns 32 of 256 batches):
  - h = X @ W.T as THREE bf16 matmul passes (Xh*Wh + Xh*Wl + Xl*Wh) where
    X = Xh + Xl is a host-side bf16 hi/lo split (same HBM bytes as fp32 X).
    h abs err ~3.5e-6 -> spike rel err ~6e-3 (CPU-sim verified); 3x cheaper
    on the PE than fp32's 4-cycle/row mode. fp32r (1 cyc/row, 11-bit
    mantissa) was measured too imprecise naked (rel 3.9e-2).
  - Input arrives t-major [T, 32b, 2, 768pad] bf16; per 4-t chunk one DMA
    lands [128=(4t x 32b), 4 x 768B lines] in SBUF (the 4-line split makes
    the DMA emit 16 descriptor packets -> uses all 16 SDMA engines; a
    single 3072B line per partition only engages 4).
  - PE transposes the chunk (bf16) into [i, tb] K-chunks via PSUM; ACT
    copies to SBUF; 18 matmuls (3 passes x 6 K-chunks, X stationary /
    W moving 400-wide) accumulate h into PSUM [128 tb, 400 o] fp32.
    Matmuls for chunk n are emitted AFTER transposes of chunk n+1 so the
    PE never stalls on the ACT copy.
  - ACT stages h into hstg4 [p, oc, c(4 chunks), o']; each 4-chunk batch is
    scattered by 4 DMAs (one per t-row group) into the recurrence layout
    hseg [p'=(b*4+oc), t, o'] (32->128 partition fan-out, 400B lines).
  - The recurrence runs as 2 fused custom DVE ops per timestep, emitted
    incrementally right after each reshuffle flush so the serial chain
    hides under the PE span:
      mem' = (mem <= 1) ? (beta*mem + syn) : 0   (MEMSTEP)
      syn' = alpha*syn + h_t                     (SYNSTEP)
    (A per-segment tensor_tensor_scan for syn was ~90 us slower end-to-end:
    its 100-instruction bursts serialized into a long post-PE tail.)
  - spikes = (mem ring > 1) in 20-t batches on DVE; written straight to
    HBM by 4 stride-4-partition DMAs issued from the GPSIMD SWDGE queue --
    keeping these spike-gated DMAs off the SP HWDGE FIFO so input/reshuffle
    DMAs are never stuck behind them.
"""

import numpy as np
import ml_dtypes

ALPHA = 0.9
BETA = 0.85

B_FULL, T_FULL, I_FULL, O_FULL = 256, 500, 700, 400
IPAD = 768
NCORES = 8

_CACHE = {}


# --------------------------------------------------------------------------- #
# Custom DVE op: one fused membrane update step.
#   out = select(mem <= 1, beta*mem + syn, 0)
# --------------------------------------------------------------------------- #
def _register_custom_op(name, spec_fn):
    import concourse.dve_ops as dvo

    for op in dvo.OPS:
        if op.name == name:
            return op

    spec = spec_fn()

    def _append(op):
        dvo.OPS.append(op)
        dvo.CUSTOM_DVE_SPECS[op.name] = op.spec
        dvo._SUB_OPCODE_FOR_NAME[op.name] = dvo._CUSTOM_DVE_ROW_BASE + len(dvo.OPS) - 1

    # Two-phase registration: learn the uops shas from the pin-check error.
    import re as _re

    probe = dvo.DveOp(name, spec, subdim=False, uops_sha={})
    _append(probe)
    shas = {}
    for ver in ("v3", "v4"):
        try:
            probe.compile(ver)
            shas[ver] = probe.uops_sha[ver]
        except ValueError as e:
            m = _re.search(r'uops_sha\["(v\d)"\]="([0-9a-f]+)"', str(e))
            shas[m.group(1)] = m.group(2)
    dvo.OPS.remove(probe)
    del dvo._SUB_OPCODE_FOR_NAME[probe.name]
    final = dvo.DveOp(name, spec, subdim=False, uops_sha=shas)
    _append(final)
    return final


def _register_memstep():
    from concourse.dve_spec import Spec, Src0, Src1, C0, Zero, One, select

    def _mk():
        def _ref(in0, in1, s0, s1, imm2):
            a = (in0.astype(np.float32) * np.float32(s0) + in1).astype(np.float32)
            return np.where(in0 <= 1.0, a, np.float32(0.0)).astype(np.float32)

        return Spec(body=select(Src0 <= One, Src0 * C0 + Src1, Zero), reference=_ref)

    return _register_custom_op("SNN_MEMSTEP_ANT", _mk)


def _register_synstep():
    from concourse.dve_spec import Spec, Src0, Src1, C0

    def _mk():
        def _ref(in0, in1, s0, s1, imm2):
            return (in0.astype(np.float32) * np.float32(s0) + in1).astype(np.float32)

        return Spec(body=Src0 * C0 + Src1, reference=_ref)

    return _register_custom_op("SNN_SYNSTEP_ANT", _mk)


# --------------------------------------------------------------------------- #
# Program builder (per-core SPMD program).
# --------------------------------------------------------------------------- #
def build_program(B_L, T, I, O, seg_lens=None, memk=20):
    import concourse.bass as bass
    import concourse.bacc as bacc
    import concourse.mybir as mybir
    import concourse.tile as tile

    MEMSTEP = _register_memstep()
    SYNSTEP = _register_synstep()

    P = 128
    TC = P // B_L                      # timesteps per matmul chunk (4)
    assert B_L * TC == P
    NCH = T // TC                      # matmul chunks (125)
    assert NCH * TC == T
    NK = IPAD // P                     # K-chunks (6)
    OC = 4                             # o'-groups (4 x 100)
    OP = O // OC                       # o' lanes per partition (100)
    MEMK = memk
    if seg_lens is None:
        seg_lens = [100] * 5 if T == 500 else [T]
    assert sum(seg_lens) == T
    assert all(sl % TC == 0 and sl % MEMK == 0 for sl in seg_lens)
    SEG_STARTS = [int(v) for v in np.cumsum([0] + seg_lens)]
    SEG_MAX = max(seg_lens)
    NSEG = len(seg_lens)

    f32 = mybir.dt.float32
    bf16 = mybir.dt.bfloat16

    nc = bacc.Bacc(
        "TRN2",
        target_bir_lowering=False,
        debug=False,
        enable_asserts=False,
        num_devices=1,
    )

    x2_d = nc.dram_tensor("x2", [T, B_L, 2, IPAD], bf16, kind="ExternalInput").ap()
    wh_d = nc.dram_tensor("wh", [O, IPAD], bf16, kind="ExternalInput").ap()
    wl_d = nc.dram_tensor("wl", [O, IPAD], bf16, kind="ExternalInput").ap()
    id_d = nc.dram_tensor("ident", [P, P], bf16, kind="ExternalInput").ap()
    out_d = nc.dram_tensor("out", [B_L, T, O], f32, kind="ExternalOutput").ap()

    with tile.TileContext(nc) as tc:
        with (
            tc.tile_pool(name="persist", bufs=1) as pp,
            tc.tile_pool(name="xn", bufs=4) as xnp,
            tc.tile_pool(name="xt", bufs=4) as xtp,
            tc.tile_pool(name="stage", bufs=2) as stp,
            tc.tile_pool(name="hstg", bufs=3) as hsp,
            tc.tile_pool(name="xt_ps", bufs=2, space=bass.MemorySpace.PSUM) as xtpp,
            tc.tile_pool(name="h_ps", bufs=3, space=bass.MemorySpace.PSUM) as hpp,
        ):
            # ---------------- persistent tiles ----------------
            ident = pp.tile([P, P], bf16)
            nc.sync.dma_start(ident[:, :], id_d[:, :])
            wt_h = pp.tile([P, NK, O], bf16)           # [i_sub, k, o]
            wt_l = pp.tile([P, NK, O], bf16)
            syn = pp.tile([P, OP], f32)
            nc.vector.memset(syn[:, :], 0.0)
            ring = pp.tile([P, MEMK + 1, OP], f32)
            nc.vector.memset(ring[:, 0, :], 0.0)
            hseg = [
                pp.tile([P, SEG_MAX, OP], f32, name=f"hseg{i}", tag=f"hseg{i}")
                for i in range(2)
            ]

            # ---------------- W -> Wt (one-time) ----------------
            with (
                tc.tile_pool(name="wsetup", bufs=1) as wsp,
                tc.tile_pool(name="w_ps", bufs=1, space=bass.MemorySpace.PSUM) as wpp,
            ):
                for w_d, wt in ((wh_d, wt_h), (wl_d, wt_l)):
                    w_stage = wsp.tile([P, OC, IPAD], bf16, tag="wstage")
                    for c in range(OC):
                        pc = min(P, O - c * P)
                        if pc <= 0:
                            break
                        nc.sync.dma_start(
                            w_stage[0:pc, c, :], w_d[c * P:c * P + pc, :]
                        )
                    for k in range(NK):
                        w_ps = wpp.tile([P, O], bf16, tag="w_ps")
                        for c in range(OC):
                            pc = min(P, O - c * P)
                            if pc <= 0:
                                break
                            nc.tensor.transpose(
                                w_ps[0:P, c * P:c * P + pc],
                                w_stage[0:pc, c, k * P:(k + 1) * P],
                                ident[0:pc, 0:pc],
                            )
                        nc.scalar.copy(wt[:, k, :], w_ps[:, :])

            # ---------------- helpers ----------------
            def emit_steps(gt0, gt1):
                # membrane + synapse steps for global t in [gt0, gt1), emitted
                # incrementally right after the reshuffle flush that made the
                # needed h columns available — the serial DVE chain then runs
                # concurrently with the rest of the pipeline.
                for t in range(gt0, gt1):
                    s = seg_of(t)
                    tl = t - SEG_STARTS[s]
                    j = t % MEMK
                    # mem' = select(mem <= 1, beta*mem + syn_{t-1}, 0)
                    nc.vector._custom_dve(
                        MEMSTEP,
                        out=ring[:, j + 1, :],
                        in0=ring[:, j, :],
                        in1=syn[:, :],
                        s0=BETA,
                    )
                    if j == MEMK - 1:
                        tb0 = t - (MEMK - 1)
                        # spikes on DVE: stage[p=(b,oc), t(20), o'(100)]
                        stage = stp.tile([P, MEMK, OP], f32, tag="stage")
                        nc.vector.tensor_scalar(
                            stage[:, :, :],
                            ring[:, 0:MEMK, :],
                            1.0,
                            None,
                            op0=mybir.AluOpType.is_gt,
                        )
                        nc.vector.tensor_copy(ring[:, 0, :], ring[:, MEMK, :])
                        # direct out via the idle GPSIMD (SWDGE) queue: keeps
                        # spike-gated DMAs off the SP FIFO so the next
                        # segment's input DMAs are never stuck behind them
                        for oc in range(OC):
                            nc.gpsimd.dma_start(
                                out_d[:, tb0:tb0 + MEMK, oc * OP:(oc + 1) * OP],
                                stage[oc::OC, :, :],
                            )
                    # syn' = alpha*syn + h_t  (skip for the final step)
                    if t < T - 1:
                        nc.vector._custom_dve(
                            SYNSTEP,
                            out=syn[:, :],
                            in0=syn[:, :],
                            in1=hseg[s % 2][:, tl, :],
                            s0=ALPHA,
                        )

            # ---------------- main pipeline ----------------
            x_tb = x2_d                            # [T, B_L, 2, IPAD] t-major
            RB = 4                                 # chunks per reshuffle batch
            hstg4 = None

            def seg_of(t):
                for si in range(NSEG):
                    if t < SEG_STARTS[si + 1]:
                        return si
                raise AssertionError

            bstate = {"c": 0, "tlb": 0}

            def consume(n, xt):
                # matmuls + staging + reshuffle for a chunk
                nonlocal hstg4
                t0 = n * TC
                s = seg_of(t0)
                tl0 = t0 - SEG_STARTS[s]
                ns = tl0 // TC                     # chunk index within segment
                cps = seg_lens[s] // TC            # chunks in this segment
                c = bstate["c"]                    # position within batch
                if c == 0:
                    bstate["tlb"] = tl0

                # -- matmul: h_ps[tb, o] = Xh Wh + Xh Wl + Xl Wh
                h_ps = hpp.tile([P, O], f32, tag="h_ps")
                passes = ((0, wt_h), (0, wt_l), (1, wt_h))
                npass = len(passes)
                for pi, (a, wt) in enumerate(passes):
                    for k in range(NK):
                        nc.tensor.matmul(
                            h_ps[:, :],
                            xt[:, a, k * P:(k + 1) * P],
                            wt[:, k, :],
                            start=(pi == 0 and k == 0),
                            stop=(pi == npass - 1 and k == NK - 1),
                        )

                # -- stage h PSUM->SBUF (ACT) into [p, oc, c, o'] batch tile
                if c == 0:
                    hstg4 = hsp.tile([P, OC, RB, OP], f32, tag="hstg")
                nc.scalar.copy(
                    hstg4[:, :, c, :],
                    h_ps[:, :].rearrange("p (oc o) -> p oc o", oc=OC),
                )

                # -- end of batch (or segment, or near the kernel's end where
                # per-chunk flushing keeps the final DVE tail short)
                if c == RB - 1 or ns == cps - 1:
                    rb = c + 1
                    tlb = bstate["tlb"]            # first t of batch in segment
                    bstate["c"] = 0
                    hb = hseg[s % 2]
                    for tau in range(TC):
                        if rb == RB:
                            nc.sync.dma_start(
                                hb[:, tlb + tau:tlb + tau + (rb - 1) * TC + 1:TC, :],
                                hstg4[tau * 32:(tau + 1) * 32, :, 0:rb, :],
                            )
                        else:
                            # partial tail batch: per-chunk DMAs (AP balance
                            # can't express the strided scatter in <=3 dims)
                            for cc in range(rb):
                                nc.sync.dma_start(
                                    hb[:, tlb + cc * TC + tau, :],
                                    hstg4[tau * 32:(tau + 1) * 32, :, cc, :],
                                )
                    gt0 = SEG_STARTS[s] + tlb
                    emit_steps(gt0, gt0 + rb * TC)
                else:
                    bstate["c"] = c + 1

            IH = IPAD // 2                 # 384
            IHP = IH + 16                  # padded half-line (400 elems)
            # two-chunk pairing halves the number of PE transpose<->matmul
            # transitions (each costs a ~150ns cross-engine sem wait)
            prev_pair = []
            pend = []
            for n in range(NCH):
                t0 = n * TC

                # -- load X chunk [128=(4t x 32b), 2, IPAD] (hi+lo interleaved).
                # The tile splits each partition's 3072B into 4 x 768B lines
                # (+32B pad) so the DMA emits 512 descriptors -> 16 packets ->
                # spreads across all 16 SDMA engines instead of 4.
                xn = xnp.tile([P, 4, IHP], bf16, tag="xn")
                nc.sync.dma_start(
                    xn[:, :, 0:IH],
                    x_tb[t0:t0 + TC, :, :, :],
                )

                # -- PE transpose to [i, tb] chunks (bf16, into one PSUM tile)
                xt_ps = xtpp.tile([P, 2, NK * P], bf16, tag="xt_ps")
                for a in range(2):
                    for k in range(NK):
                        m = a * 2 + k // 3
                        c0 = (k % 3) * P
                        nc.tensor.transpose(
                            xt_ps[:, a, k * P:(k + 1) * P],
                            xn[:, m, c0:c0 + P],
                            ident[:, :],
                        )
                xt = xtp.tile([P, 2, NK * P], bf16, tag="xt")
                nc.scalar.copy(xt[:, :, :], xt_ps[:, :, :])

                pend.append((n, xt))
                if n % 2 == 1:
                    for item in prev_pair:
                        consume(*item)
                    prev_pair = pend
                    pend = []
            for item in prev_pair + pend:
                consume(*item)

    nc.compile()
    return nc, {"B_L": B_L, "T": T, "I": I, "O": O}


# --------------------------------------------------------------------------- #
# Host-side entry point
# --------------------------------------------------------------------------- #
def split_pad(a, pad_to):
    bf = ml_dtypes.bfloat16
    hi = a.astype(bf)
    lo = (a - hi.astype(np.float32)).astype(bf)
    if a.shape[-1] != pad_to:
        pw = [(0, 0)] * (a.ndim - 1) + [(0, pad_to - a.shape[-1])]
        hi = np.pad(hi, pw)
        lo = np.pad(lo, pw)
    return np.ascontiguousarray(hi), np.ascontiguousarray(lo)


def build_x2(x):
    """Interleave the bf16 hi/lo split, t-major: [T, B, 2, IPAD]."""
    bf = ml_dtypes.bfloat16
    B, T, I = x.shape
    x2 = np.zeros((T, B, 2, IPAD), dtype=bf)
    hi = x.astype(bf)
    x2[:, :, 0, :I] = np.swapaxes(hi, 0, 1)
    x2[:, :, 1, :I] = np.swapaxes((x - hi.astype(np.float32)).astype(bf), 0, 1)
    return x2


def kernel(inputs: np.ndarray, W: np.ndarray, nb_steps) -> np.ndarray:
    from concourse.bass_utils import run_bass_kernel_spmd

    B, T, I = inputs.shape
    O = W.shape[0]
    assert (B, T, I, O) == (B_FULL, T_FULL, I_FULL, O_FULL), (B, T, I, O)
    assert int(nb_steps) == T

    key = (B, T, I, O)
    if key not in _CACHE:
        _CACHE[key] = build_program(B // NCORES, T, I, O)
    nc, meta = _CACHE[key]

    bf = ml_dtypes.bfloat16
    x = np.ascontiguousarray(inputs, dtype=np.float32)
    w = np.ascontiguousarray(W, dtype=np.float32)

    x2 = build_x2(x)
    wh, wl = split_pad(w, IPAD)
    ident = np.eye(128, dtype=bf)

    B_L = B // NCORES
    in_maps = [
        {
            "x2": np.ascontiguousarray(x2[:, c * B_L:(c + 1) * B_L]),
            "wh": wh,
            "wl": wl,
            "ident": ident,
        }
        for c in range(NCORES)
    ]
    results = run_bass_kernel_spmd(nc, in_maps, core_ids=list(range(NCORES)))
    outs = [r["out"] for r in results.results]
    return np.concatenate(outs, axis=0)


# revision 58
# speedup vs baseline: 1.0685x; 1.0050x over previous
"""SNN recurrent layer (Linear + leaky-integrate-and-fire scan) on 8 trn2 NeuronCores.

~524 us HW exec (vs 1492 us fp32 baseline), rel err ~7.6e-3 (gate 2e-2).

Strategy (pure data parallel over batch; each core owns 32 of 256 batches):
  - h = X @ W.T as THREE bf16 matmul passes (Xh*Wh + Xh*Wl + Xl*Wh) where
    X = Xh + Xl is a host-side bf16 hi/lo split (same HBM bytes as fp32 X).
    h abs err ~3.5e-6 -> spike rel err ~6e-3 (CPU-sim verified); 3x cheaper
    on the PE than fp32's 4-cycle/row mode. fp32r (1 cyc/row, 11-bit
    mantissa) was measured too imprecise naked (rel 3.9e-2).
  - Input arrives t-major [T, 32b, 2, 768pad] bf16; per 4-t chunk one DMA
    lands [128=(4t x 32b), 4 x 768B lines] in SBUF (the 4-line split makes
    the DMA emit 16 descriptor packets -> uses all 16 SDMA engines; a
    single 3072B line per partition only engages 4).
  - PE transposes the chunk (bf16) into [i, tb] K-chunks via PSUM; ACT
    copies to SBUF; 18 matmuls (3 passes x 6 K-chunks, X stationary /
    W moving 400-wide) accumulate h into PSUM [128 tb, 400 o] fp32.
    Matmuls for chunk n are emitted AFTER transposes of chunk n+1 so the
    PE never stalls on the ACT copy.
  - ACT stages h into hstg4 [p, oc, c(4 chunks), o']; each 4-chunk batch is
    scattered by 4 DMAs (one per t-row group) into the recurrence layout
    hseg [p'=(b*4+oc), t, o'] (32->128 partition fan-out, 400B lines).
  - The recurrence runs as 2 fused custom DVE ops per timestep, emitted
    incrementally right after each reshuffle flush so the serial chain
    hides under the PE span:
      mem' = (mem <= 1) ? (beta*mem + syn) : 0   (MEMSTEP)
      syn' = alpha*syn + h_t                     (SYNSTEP)
    (A per-segment tensor_tensor_scan for syn was ~90 us slower end-to-end:
    its 100-instruction bursts serialized into a long post-PE tail.)
  - spikes = (mem ring > 1) in 20-t batches on DVE; written straight to
    HBM by 4 stride-4-partition DMAs issued from the GPSIMD SWDGE queue --
    keeping these spike-gated DMAs off the SP HWDGE FIFO so input/reshuffle
    DMAs are never stuck behind them.
"""

import numpy as np
import ml_dtypes

ALPHA = 0.9
BETA = 0.85

B_FULL, T_FULL, I_FULL, O_FULL = 256, 500, 700, 400
IPAD = 768
NCORES = 8

_CACHE = {}


# --------------------------------------------------------------------------- #
# Custom DVE op: one fused membrane update step.
#   out = select(mem <= 1, beta*mem + syn, 0)
# --------------------------------------------------------------------------- #
def _register_custom_op(name, spec_fn):
    import concourse.dve_ops as dvo

    for op in dvo.OPS:
        if op.name == name:
            return op

    spec = spec_fn()

    def _append(op):
        dvo.OPS.append(op)
        dvo.CUSTOM_DVE_SPECS[op.name] = op.spec
        dvo._SUB_OPCODE_FOR_NAME[op.name] = dvo._CUSTOM_DVE_ROW_BASE + len(dvo.OPS) - 1

    # Two-phase registration: learn the uops shas from the pin-check error.
    import re as _re

    probe = dvo.DveOp(name, spec, subdim=False, uops_sha={})
    _append(probe)
    shas = {}
    for ver in ("v3", "v4"):
        try:
            probe.compile(ver)
            shas[ver] = probe.uops_sha[ver]
        except ValueError as e:
            m = _re.search(r'uops_sha\["(v\d)"\]="([0-9a-f]+)"', str(e))
            shas[m.group(1)] = m.group(2)
    dvo.OPS.remove(probe)
    del dvo._SUB_OPCODE_FOR_NAME[probe.name]
    final = dvo.DveOp(name, spec, subdim=False, uops_sha=shas)
    _append(final)
    return final


def _register_memstep():
    from concourse.dve_spec import Spec, Src0, Src1, C0, Zero, One, select

    def _mk():
        def _ref(in0, in1, s0, s1, imm2):
            a = (in0.astype(np.float32) * np.float32(s0) + in1).astype(np.float32)
            return np.where(in0 <= 1.0, a, np.float32(0.0)).astype(np.float32)

        return Spec(body=select(Src0 <= One, Src0 * C0 + Src1, Zero), reference=_ref)

    return _register_custom_op("SNN_MEMSTEP_ANT", _mk)


def _register_synstep():
    from concourse.dve_spec import Spec, Src0, Src1, C0

    def _mk():
        def _ref(in0, in1, s0, s1, imm2):
            return (in0.astype(np.float32) * np.float32(s0) + in1).astype(np.float32)

        return Spec(body=Src0 * C0 + Src1, reference=_ref)

    return _register_custom_op("SNN_SYNSTEP_ANT", _mk)


# --------------------------------------------------------------------------- #
# Program builder (per-core SPMD program).
# --------------------------------------------------------------------------- #
def build_program(B_L, T, I, O, seg_lens=None, memk=20):
    import concourse.bass as bass
    import concourse.bacc as bacc
    import concourse.mybir as mybir
    import concourse.tile as tile

    MEMSTEP = _register_memstep()
    SYNSTEP = _register_synstep()

    P = 128
    TC = P // B_L                      # timesteps per matmul chunk (4)
    assert B_L * TC == P
    NCH = T // TC                      # matmul chunks (125)
    assert NCH * TC == T
    NK = IPAD // P                     # K-chunks (6)
    OC = 4                             # o'-groups (4 x 100)
    OP = O // OC                       # o' lanes per partition (100)
    MEMK = memk
    if seg_lens is None:
        seg_lens = [100] * 5 if T == 500 else [T]
    assert sum(seg_lens) == T
    assert all(sl % TC == 0 and sl % MEMK == 0 for sl in seg_lens)
    SEG_STARTS = [int(v) for v in np.cumsum([0] + seg_lens)]
    SEG_MAX = max(seg_lens)
    NSEG = len(seg_lens)

    f32 = mybir.dt.float32
    bf16 = mybir.dt.bfloat16

    nc = bacc.Bacc(
        "TRN2",
        target_bir_lowering=False,
        debug=False,
        enable_asserts=False,
        num_devices=1,
    )

    x2_d = nc.dram_tensor("x2", [T, B_L, 2, IPAD], bf16, kind="ExternalInput").ap()
    wh_d = nc.dram_tensor("wh", [O, IPAD], bf16, kind="ExternalInput").ap()
    wl_d = nc.dram_tensor("wl", [O, IPAD], bf16, kind="ExternalInput").ap()
    id_d = nc.dram_tensor("ident", [P, P], bf16, kind="ExternalInput").ap()
    out_d = nc.dram_tensor("out", [B_L, T, O], f32, kind="ExternalOutput").ap()

    with tile.TileContext(nc) as tc:
        with (
            tc.tile_pool(name="persist", bufs=1) as pp,
            tc.tile_pool(name="xn", bufs=4) as xnp,
            tc.tile_pool(name="xt", bufs=3) as xtp,
            tc.tile_pool(name="stage", bufs=2) as stp,
            tc.tile_pool(name="hstg", bufs=3) as hsp,
            tc.tile_pool(name="xt_ps", bufs=2, space=bass.MemorySpace.PSUM) as xtpp,
            tc.tile_pool(name="h_ps", bufs=3, space=bass.MemorySpace.PSUM) as hpp,
        ):
            # ---------------- persistent tiles ----------------
            ident = pp.tile([P, P], bf16)
            nc.sync.dma_start(ident[:, :], id_d[:, :])
            wt_h = pp.tile([P, NK, O], bf16)           # [i_sub, k, o]
            wt_l = pp.tile([P, NK, O], bf16)
            syn = pp.tile([P, OP], f32)
            nc.vector.memset(syn[:, :], 0.0)
            ring = pp.tile([P, MEMK + 1, OP], f32)
            nc.vector.memset(ring[:, 0, :], 0.0)
            hseg = [
                pp.tile([P, SEG_MAX, OP], f32, name=f"hseg{i}", tag=f"hseg{i}")
                for i in range(2)
            ]

            # ---------------- W -> Wt (one-time) ----------------
            with (
                tc.tile_pool(name="wsetup", bufs=1) as wsp,
                tc.tile_pool(name="w_ps", bufs=1, space=bass.MemorySpace.PSUM) as wpp,
            ):
                for w_d, wt in ((wh_d, wt_h), (wl_d, wt_l)):
                    w_stage = wsp.tile([P, OC, IPAD], bf16, tag="wstage")
                    for c in range(OC):
                        pc = min(P, O - c * P)
                        if pc <= 0:
                            break
                        nc.sync.dma_start(
                            w_stage[0:pc, c, :], w_d[c * P:c * P + pc, :]
                        )
                    for k in range(NK):
                        w_ps = wpp.tile([P, O], bf16, tag="w_ps")
                        for c in range(OC):
                            pc = min(P, O - c * P)
                            if pc <= 0:
                                break
                            nc.tensor.transpose(
                                w_ps[0:P, c * P:c * P + pc],
                                w_stage[0:pc, c, k * P:(k + 1) * P],
                                ident[0:pc, 0:pc],
                            )
                        nc.scalar.copy(wt[:, k, :], w_ps[:, :])

            # ---------------- helpers ----------------
            def emit_steps(gt0, gt1):
                # membrane + synapse steps for global t in [gt0, gt1), emitted
                # incrementally right after the reshuffle flush that made the
                # needed h columns available — the serial DVE chain then runs
                # concurrently with the rest of the pipeline.
                for t in range(gt0, gt1):
                    s = seg_of(t)
                    tl = t - SEG_STARTS[s]
                    j = t % MEMK
                    # mem' = select(mem <= 1, beta*mem + syn_{t-1}, 0)
                    nc.vector._custom_dve(
                        MEMSTEP,
                        out=ring[:, j + 1, :],
                        in0=ring[:, j, :],
                        in1=syn[:, :],
                        s0=BETA,
                    )
                    if j == MEMK - 1:
                        tb0 = t - (MEMK - 1)
                        # spikes on DVE: stage[p=(b,oc), t(20), o'(100)]
                        stage = stp.tile([P, MEMK, OP], f32, tag="stage")
                        nc.vector.tensor_scalar(
                            stage[:, :, :],
                            ring[:, 0:MEMK, :],
                            1.0,
                            None,
                            op0=mybir.AluOpType.is_gt,
                        )
                        nc.vector.tensor_copy(ring[:, 0, :], ring[:, MEMK, :])
                        # direct out via the idle GPSIMD (SWDGE) queue: keeps
                        # spike-gated DMAs off the SP FIFO so the next
                        # segment's input DMAs are never stuck behind them
                        for oc in range(OC):
                            nc.gpsimd.dma_start(
                                out_d[:, tb0:tb0 + MEMK, oc * OP:(oc + 1) * OP],
                                stage[oc::OC, :, :],
                            )
                    # syn' = alpha*syn + h_t  (skip for the final step)
                    if t < T - 1:
                        nc.vector._custom_dve(
                            SYNSTEP,
                            out=syn[:, :],
                            in0=syn[:, :],
                            in1=hseg[s % 2][:, tl, :],
                            s0=ALPHA,
                        )

            # ---------------- main pipeline ----------------
            x_tb = x2_d                            # [T, B_L, 2, IPAD] t-major
            RB = 4                                 # chunks per reshuffle batch
            hstg4 = None

            def seg_of(t):
                for si in range(NSEG):
                    if t < SEG_STARTS[si + 1]:
                        return si
                raise AssertionError

            bstate = {"c": 0, "tlb": 0}

            def consume(n, xt):
                # matmuls + staging + reshuffle for a chunk
                nonlocal hstg4
                t0 = n * TC
                s = seg_of(t0)
                tl0 = t0 - SEG_STARTS[s]
                ns = tl0 // TC                     # chunk index within segment
                cps = seg_lens[s] // TC            # chunks in this segment
                c = bstate["c"]                    # position within batch
                if c == 0:
                    bstate["tlb"] = tl0

                # -- matmul: h_ps[tb, o] = Xh Wh + Xh Wl + Xl Wh
                h_ps = hpp.tile([P, O], f32, tag="h_ps")
                passes = ((0, wt_h), (0, wt_l), (1, wt_h))
                npass = len(passes)
                for pi, (a, wt) in enumerate(passes):
                    for k in range(NK):
                        nc.tensor.matmul(
                            h_ps[:, :],
                            xt[:, a, k * P:(k + 1) * P],
                            wt[:, k, :],
                            start=(pi == 0 and k == 0),
                            stop=(pi == npass - 1 and k == NK - 1),
                        )

                # -- stage h PSUM->SBUF (ACT) into [p, oc, c, o'] batch tile
                if c == 0:
                    hstg4 = hsp.tile([P, OC, RB, OP], f32, tag="hstg")
                nc.scalar.copy(
                    hstg4[:, :, c, :],
                    h_ps[:, :].rearrange("p (oc o) -> p oc o", oc=OC),
                )

                # -- end of batch (or segment, or near the kernel's end where
                # per-chunk flushing keeps the final DVE tail short)
                if c == RB - 1 or ns == cps - 1:
                    rb = c + 1
                    tlb = bstate["tlb"]            # first t of batch in segment
                    bstate["c"] = 0
                    hb = hseg[s % 2]
                    for tau in range(TC):
                        if rb == RB:
                            nc.sync.dma_start(
                                hb[:, tlb + tau:tlb + tau + (rb - 1) * TC + 1:TC, :],
                                hstg4[tau * 32:(tau + 1) * 32, :, 0:rb, :],
                            )
                        else:
                            # partial tail batch: per-chunk DMAs (AP balance
                            # can't express the strided scatter in <=3 dims)
                            for cc in range(rb):
                                nc.sync.dma_start(
                                    hb[:, tlb + cc * TC + tau, :],
                                    hstg4[tau * 32:(tau + 1) * 32, :, cc, :],
                                )
                    gt0 = SEG_STARTS[s] + tlb
                    emit_steps(gt0, gt0 + rb * TC)
                else:
                    bstate["c"] = c + 1

            IH = IPAD // 2                 # 384
            IHP = IH + 16                  # padded half-line (400 elems)
            pending = None
            for n in range(NCH):
                t0 = n * TC

                # -- load X chunk [128=(4t x 32b), 2, IPAD] (hi+lo interleaved).
                # The tile splits each partition's 3072B into 4 x 768B lines
                # (+32B pad) so the DMA emits 512 descriptors -> 16 packets ->
                # spreads across all 16 SDMA engines instead of 4.
                xn = xnp.tile([P, 4, IHP], bf16, tag="xn")
                nc.sync.dma_start(
                    xn[:, :, 0:IH],
                    x_tb[t0:t0 + TC, :, :, :],
                )

                # -- PE transpose to [i, tb] chunks (bf16, into one PSUM tile)
                xt_ps = xtpp.tile([P, 2, NK * P], bf16, tag="xt_ps")
                for a in range(2):
                    for k in range(NK):
                        m = a * 2 + k // 3
                        c0 = (k % 3) * P
                        nc.tensor.transpose(
                            xt_ps[:, a, k * P:(k + 1) * P],
                            xn[:, m, c0:c0 + P],
                            ident[:, :],
                        )
                xt = xtp.tile([P, 2, NK * P], bf16, tag="xt")
                nc.scalar.copy(xt[:, :, :], xt_ps[:, :, :])

                if pending is not None:
                    consume(*pending)
                pending = (n, xt)
            consume(*pending)

    nc.compile()
    return nc, {"B_L": B_L, "T": T, "I": I, "O": O}


# --------------------------------------------------------------------------- #
# Host-side entry point
# --------------------------------------------------------------------------- #
def split_pad(a, pad_to):
    bf = ml_dtypes.bfloat16
    hi = a.astype(bf)
    lo = (a - hi.astype(np.float32)).astype(bf)
    if a.shape[-1] != pad_to:
        pw = [(0, 0)] * (a.ndim - 1) + [(0, pad_to - a.shape[-1])]
        hi = np.pad(hi, pw)
        lo = np.pad(lo, pw)
    return np.ascontiguousarray(hi), np.ascontiguousarray(lo)


def build_x2(x):
    """Interleave the bf16 hi/lo split, t-major: [T, B, 2, IPAD]."""
    bf = ml_dtypes.bfloat16
    B, T, I = x.shape
    x2 = np.zeros((T, B, 2, IPAD), dtype=bf)
    hi = x.astype(bf)
    x2[:, :, 0, :I] = np.swapaxes(hi, 0, 1)
    x2[:, :, 1, :I] = np.swapaxes((x - hi.astype(np.float32)).astype(bf), 0, 1)
    return x2


def kernel(inputs: np.ndarray, W: np.ndarray, nb_steps) -> np.ndarray:
    from concourse.bass_utils import run_bass_kernel_spmd

    B, T, I = inputs.shape
    O = W.shape[0]
    assert (B, T, I, O) == (B_FULL, T_FULL, I_FULL, O_FULL), (B, T, I, O)
    assert int(nb_steps) == T

    key = (B, T, I, O)
    if key not in _CACHE:
        _CACHE[key] = build_program(B // NCORES, T, I, O)
    nc, meta = _CACHE[key]

    bf = ml_dtypes.bfloat16
    x = np.ascontiguousarray(inputs, dtype=np.float32)
    w = np.ascontiguousarray(W, dtype=np.float32)

    x2 = build_x2(x)
    wh, wl = split_pad(w, IPAD)
    ident = np.eye(128, dtype=bf)

    B_L = B // NCORES
    in_maps = [
        {
            "x2": np.ascontiguousarray(x2[:, c * B_L:(c + 1) * B_L]),
            "wh": wh,
            "wl": wl,
            "ident": ident,
        }
        for c in range(NCORES)
    ]
    results = run_bass_kernel_spmd(nc, in_maps, core_ids=list(range(NCORES)))
    outs = [r["out"] for r in results.results]
    return np.concatenate(outs, axis=0)


# revision 67
# speedup vs baseline: 1.0741x; 1.0053x over previous
"""SNN recurrent layer (Linear + leaky-integrate-and-fire scan) on 8 trn2 NeuronCores.

~524 us HW exec (vs 1492 us fp32 baseline), rel err ~7.6e-3 (gate 2e-2).

Strategy (pure data parallel over batch; each core owns 32 of 256 batches):
  - h = X @ W.T as THREE bf16 matmul passes (Xh*Wh + Xh*Wl + Xl*Wh) where
    X = Xh + Xl is a host-side bf16 hi/lo split (same HBM bytes as fp32 X).
    h abs err ~3.5e-6 -> spike rel err ~6e-3 (CPU-sim verified); 3x cheaper
    on the PE than fp32's 4-cycle/row mode. fp32r (1 cyc/row, 11-bit
    mantissa) was measured too imprecise naked (rel 3.9e-2).
  - Input arrives t-major [T, 32b, 2, 768pad] bf16; per 4-t chunk one DMA
    lands [128=(4t x 32b), 4 x 768B lines] in SBUF (the 4-line split makes
    the DMA emit 16 descriptor packets -> uses all 16 SDMA engines; a
    single 3072B line per partition only engages 4).
  - PE transposes the chunk (bf16) into [i, tb] K-chunks via PSUM; ACT
    copies to SBUF; 18 matmuls (3 passes x 6 K-chunks, X stationary /
    W moving 400-wide) accumulate h into PSUM [128 tb, 400 o] fp32.
    Matmuls for chunk n are emitted AFTER transposes of chunk n+1 so the
    PE never stalls on the ACT copy.
  - ACT stages h into hstg4 [p, oc, c(4 chunks), o']; each 4-chunk batch is
    scattered by 4 DMAs (one per t-row group) into the recurrence layout
    hseg [p'=(b*4+oc), t, o'] (32->128 partition fan-out, 400B lines).
  - The recurrence runs as 2 fused custom DVE ops per timestep, emitted
    incrementally right after each reshuffle flush so the serial chain
    hides under the PE span:
      mem' = (mem <= 1) ? (beta*mem + syn) : 0   (MEMSTEP)
      syn' = alpha*syn + h_t                     (SYNSTEP)
    (A per-segment tensor_tensor_scan for syn was ~90 us slower end-to-end:
    its 100-instruction bursts serialized into a long post-PE tail.)
  - spikes = (mem ring > 1) in 20-t batches on DVE; written straight to
    HBM by 4 stride-4-partition DMAs issued from the GPSIMD SWDGE queue --
    keeping these spike-gated DMAs off the SP HWDGE FIFO so input/reshuffle
    DMAs are never stuck behind them.
"""

import numpy as np
import ml_dtypes

ALPHA = 0.9
BETA = 0.85

B_FULL, T_FULL, I_FULL, O_FULL = 256, 500, 700, 400
IPAD = 768          # hi-pass W padding (6 x 128)
ICAT = 1408         # concat layout: [Xh(700) | Xl(700) | pad(8)] = 11 x 128
NCORES = 8

_CACHE = {}


# --------------------------------------------------------------------------- #
# Custom DVE op: one fused membrane update step.
#   out = select(mem <= 1, beta*mem + syn, 0)
# --------------------------------------------------------------------------- #
def _register_custom_op(name, spec_fn):
    import concourse.dve_ops as dvo

    for op in dvo.OPS:
        if op.name == name:
            return op

    spec = spec_fn()

    def _append(op):
        dvo.OPS.append(op)
        dvo.CUSTOM_DVE_SPECS[op.name] = op.spec
        dvo._SUB_OPCODE_FOR_NAME[op.name] = dvo._CUSTOM_DVE_ROW_BASE + len(dvo.OPS) - 1

    # Two-phase registration: learn the uops shas from the pin-check error.
    import re as _re

    probe = dvo.DveOp(name, spec, subdim=False, uops_sha={})
    _append(probe)
    shas = {}
    for ver in ("v3", "v4"):
        try:
            probe.compile(ver)
            shas[ver] = probe.uops_sha[ver]
        except ValueError as e:
            m = _re.search(r'uops_sha\["(v\d)"\]="([0-9a-f]+)"', str(e))
            shas[m.group(1)] = m.group(2)
    dvo.OPS.remove(probe)
    del dvo._SUB_OPCODE_FOR_NAME[probe.name]
    final = dvo.DveOp(name, spec, subdim=False, uops_sha=shas)
    _append(final)
    return final


def _register_memstep():
    from concourse.dve_spec import Spec, Src0, Src1, C0, Zero, One, select

    def _mk():
        def _ref(in0, in1, s0, s1, imm2):
            a = (in0.astype(np.float32) * np.float32(s0) + in1).astype(np.float32)
            return np.where(in0 <= 1.0, a, np.float32(0.0)).astype(np.float32)

        return Spec(body=select(Src0 <= One, Src0 * C0 + Src1, Zero), reference=_ref)

    return _register_custom_op("SNN_MEMSTEP_ANT", _mk)


def _register_synstep():
    from concourse.dve_spec import Spec, Src0, Src1, C0

    def _mk():
        def _ref(in0, in1, s0, s1, imm2):
            return (in0.astype(np.float32) * np.float32(s0) + in1).astype(np.float32)

        return Spec(body=Src0 * C0 + Src1, reference=_ref)

    return _register_custom_op("SNN_SYNSTEP_ANT", _mk)


# --------------------------------------------------------------------------- #
# Program builder (per-core SPMD program).
# --------------------------------------------------------------------------- #
def build_program(B_L, T, I, O, seg_lens=None, memk=20):
    import concourse.bass as bass
    import concourse.bacc as bacc
    import concourse.mybir as mybir
    import concourse.tile as tile

    MEMSTEP = _register_memstep()
    SYNSTEP = _register_synstep()

    P = 128
    TC = P // B_L                      # timesteps per matmul chunk (4)
    assert B_L * TC == P
    NCH = T // TC                      # matmul chunks (125)
    assert NCH * TC == T
    NK = IPAD // P                     # hi-pass K-chunks (6)
    NKC = ICAT // P                    # concat-pass K-chunks (11)
    OC = 4                             # o'-groups (4 x 100)
    OP = O // OC                       # o' lanes per partition (100)
    MEMK = memk
    if seg_lens is None:
        seg_lens = [100] * 5 if T == 500 else [T]
    assert sum(seg_lens) == T
    assert all(sl % TC == 0 and sl % MEMK == 0 for sl in seg_lens)
    SEG_STARTS = [int(v) for v in np.cumsum([0] + seg_lens)]
    SEG_MAX = max(seg_lens)
    NSEG = len(seg_lens)

    f32 = mybir.dt.float32
    bf16 = mybir.dt.bfloat16

    nc = bacc.Bacc(
        "TRN2",
        target_bir_lowering=False,
        debug=False,
        enable_asserts=False,
        num_devices=1,
    )

    x2_d = nc.dram_tensor("x2", [T, B_L, ICAT], bf16, kind="ExternalInput").ap()
    wh_d = nc.dram_tensor("wh", [O, IPAD], bf16, kind="ExternalInput").ap()
    wc_d = nc.dram_tensor("wc", [O, ICAT], bf16, kind="ExternalInput").ap()
    id_d = nc.dram_tensor("ident", [P, P], bf16, kind="ExternalInput").ap()
    out_d = nc.dram_tensor("out", [B_L, T, O], f32, kind="ExternalOutput").ap()

    with tile.TileContext(nc) as tc:
        with (
            tc.tile_pool(name="persist", bufs=1) as pp,
            tc.tile_pool(name="xn", bufs=4) as xnp,
            tc.tile_pool(name="xt", bufs=3) as xtp,
            tc.tile_pool(name="stage", bufs=2) as stp,
            tc.tile_pool(name="hstg", bufs=3) as hsp,
            tc.tile_pool(name="xt_ps", bufs=2, space=bass.MemorySpace.PSUM) as xtpp,
            tc.tile_pool(name="h_ps", bufs=3, space=bass.MemorySpace.PSUM) as hpp,
        ):
            # ---------------- persistent tiles ----------------
            ident = pp.tile([P, P], bf16)
            nc.sync.dma_start(ident[:, :], id_d[:, :])
            wt_h = pp.tile([P, NK, O], bf16)           # [i_sub, k, o]
            wt_c = pp.tile([P, NKC, O], bf16)
            syn = pp.tile([P, OP], f32)
            nc.vector.memset(syn[:, :], 0.0)
            ring = pp.tile([P, MEMK + 1, OP], f32)
            nc.vector.memset(ring[:, 0, :], 0.0)
            hseg = [
                pp.tile([P, SEG_MAX, OP], f32, name=f"hseg{i}", tag=f"hseg{i}")
                for i in range(2)
            ]

            # ---------------- W -> Wt (one-time) ----------------
            with (
                tc.tile_pool(name="wsetup", bufs=1) as wsp,
                tc.tile_pool(name="w_ps", bufs=1, space=bass.MemorySpace.PSUM) as wpp,
            ):
                for w_d, wt, nk in ((wh_d, wt_h, NK), (wc_d, wt_c, NKC)):
                    w_stage = wsp.tile([P, OC, ICAT], bf16, tag="wstage")
                    wwid = w_d.shape[1]
                    for c in range(OC):
                        pc = min(P, O - c * P)
                        if pc <= 0:
                            break
                        nc.sync.dma_start(
                            w_stage[0:pc, c, 0:wwid], w_d[c * P:c * P + pc, :]
                        )
                    for k in range(nk):
                        w_ps = wpp.tile([P, O], bf16, tag="w_ps")
                        for c in range(OC):
                            pc = min(P, O - c * P)
                            if pc <= 0:
                                break
                            nc.tensor.transpose(
                                w_ps[0:P, c * P:c * P + pc],
                                w_stage[0:pc, c, k * P:(k + 1) * P],
                                ident[0:pc, 0:pc],
                            )
                        nc.scalar.copy(wt[:, k, :], w_ps[:, :])

            # ---------------- helpers ----------------
            def emit_steps(gt0, gt1):
                # membrane + synapse steps for global t in [gt0, gt1), emitted
                # incrementally right after the reshuffle flush that made the
                # needed h columns available — the serial DVE chain then runs
                # concurrently with the rest of the pipeline.
                for t in range(gt0, gt1):
                    s = seg_of(t)
                    tl = t - SEG_STARTS[s]
                    j = t % MEMK
                    # mem' = select(mem <= 1, beta*mem + syn_{t-1}, 0)
                    nc.vector._custom_dve(
                        MEMSTEP,
                        out=ring[:, j + 1, :],
                        in0=ring[:, j, :],
                        in1=syn[:, :],
                        s0=BETA,
                    )
                    if j == MEMK - 1:
                        tb0 = t - (MEMK - 1)
                        # spikes on DVE: stage[p=(b,oc), t(20), o'(100)]
                        stage = stp.tile([P, MEMK, OP], f32, tag="stage")
                        nc.vector.tensor_scalar(
                            stage[:, :, :],
                            ring[:, 0:MEMK, :],
                            1.0,
                            None,
                            op0=mybir.AluOpType.is_gt,
                        )
                        nc.vector.tensor_copy(ring[:, 0, :], ring[:, MEMK, :])
                        # direct out via the idle GPSIMD (SWDGE) queue: keeps
                        # spike-gated DMAs off the SP FIFO so the next
                        # segment's input DMAs are never stuck behind them
                        for oc in range(OC):
                            nc.gpsimd.dma_start(
                                out_d[:, tb0:tb0 + MEMK, oc * OP:(oc + 1) * OP],
                                stage[oc::OC, :, :],
                            )
                    # syn' = alpha*syn + h_t  (skip for the final step)
                    if t < T - 1:
                        nc.vector._custom_dve(
                            SYNSTEP,
                            out=syn[:, :],
                            in0=syn[:, :],
                            in1=hseg[s % 2][:, tl, :],
                            s0=ALPHA,
                        )

            # ---------------- main pipeline ----------------
            x_tb = x2_d                            # [T, B_L, 2, IPAD] t-major
            RB = 4                                 # chunks per reshuffle batch
            hstg4 = None

            def seg_of(t):
                for si in range(NSEG):
                    if t < SEG_STARTS[si + 1]:
                        return si
                raise AssertionError

            bstate = {"c": 0, "tlb": 0}

            def consume(n, xt):
                # matmuls + staging + reshuffle for a chunk
                nonlocal hstg4
                t0 = n * TC
                s = seg_of(t0)
                tl0 = t0 - SEG_STARTS[s]
                ns = tl0 // TC                     # chunk index within segment
                cps = seg_lens[s] // TC            # chunks in this segment
                c = bstate["c"]                    # position within batch
                if c == 0:
                    bstate["tlb"] = tl0

                # -- matmul: h = Xh Wh  +  [Xh|Xl] [Wl;Wh]  (= + XhWl + XlWh).
                # xt holds the concat layout [Xh(700) Xl(700) pad]; the hi
                # pass reads blocks 0-5 where wt_h's zero rows [700:768)
                # cancel the Xl head; the concat pass covers all 11 blocks.
                h_ps = hpp.tile([P, O], f32, tag="h_ps")
                for k in range(NK):
                    nc.tensor.matmul(
                        h_ps[:, :],
                        xt[:, k * P:(k + 1) * P],
                        wt_h[:, k, :],
                        start=(k == 0),
                        stop=False,
                    )
                for k in range(NKC):
                    nc.tensor.matmul(
                        h_ps[:, :],
                        xt[:, k * P:(k + 1) * P],
                        wt_c[:, k, :],
                        start=False,
                        stop=(k == NKC - 1),
                    )

                # -- stage h PSUM->SBUF (ACT) into [p, oc, c, o'] batch tile
                if c == 0:
                    hstg4 = hsp.tile([P, OC, RB, OP], f32, tag="hstg")
                nc.scalar.copy(
                    hstg4[:, :, c, :],
                    h_ps[:, :].rearrange("p (oc o) -> p oc o", oc=OC),
                )

                # -- end of batch (or segment, or near the kernel's end where
                # per-chunk flushing keeps the final DVE tail short)
                if c == RB - 1 or ns == cps - 1:
                    rb = c + 1
                    tlb = bstate["tlb"]            # first t of batch in segment
                    bstate["c"] = 0
                    hb = hseg[s % 2]
                    for tau in range(TC):
                        if rb == RB:
                            nc.sync.dma_start(
                                hb[:, tlb + tau:tlb + tau + (rb - 1) * TC + 1:TC, :],
                                hstg4[tau * 32:(tau + 1) * 32, :, 0:rb, :],
                            )
                        else:
                            # partial tail batch: per-chunk DMAs (AP balance
                            # can't express the strided scatter in <=3 dims)
                            for cc in range(rb):
                                nc.sync.dma_start(
                                    hb[:, tlb + cc * TC + tau, :],
                                    hstg4[tau * 32:(tau + 1) * 32, :, cc, :],
                                )
                    gt0 = SEG_STARTS[s] + tlb
                    emit_steps(gt0, gt0 + rb * TC)
                else:
                    bstate["c"] = c + 1

            pending = None
            for n in range(NCH):
                t0 = n * TC

                # -- load X chunk [128=(4t x 32b), ICAT] as 11 x 256B lines
                # per partition (+16B pads) so the DMA emits 44 descriptor
                # packets and spreads across all 16 SDMA engines.
                xn = xnp.tile([P, NKC, P + 8], bf16, tag="xn")
                nc.sync.dma_start(
                    xn[:, :, 0:P],
                    x_tb[t0:t0 + TC, :, :],
                )

                # -- PE transpose to [i, tb] chunks (bf16, into one PSUM tile)
                xt_ps = xtpp.tile([P, NKC * P], bf16, tag="xt_ps")
                for k in range(NKC):
                    nc.tensor.transpose(
                        xt_ps[:, k * P:(k + 1) * P],
                        xn[:, k, 0:P],
                        ident[:, :],
                    )
                xt = xtp.tile([P, NKC * P], bf16, tag="xt")
                nc.scalar.copy(xt[:, :], xt_ps[:, :])

                if pending is not None:
                    consume(*pending)
                pending = (n, xt)
            consume(*pending)

    nc.compile()
    return nc, {"B_L": B_L, "T": T, "I": I, "O": O}


# --------------------------------------------------------------------------- #
# Host-side entry point
# --------------------------------------------------------------------------- #
def build_w(w):
    """wh = bf16(W) padded to IPAD; wc = [Wl(700) | Wh(700) | 0] at ICAT."""
    bf = ml_dtypes.bfloat16
    O, I = w.shape
    hi = w.astype(bf)
    lo = (w - hi.astype(np.float32)).astype(bf)
    wh = np.zeros((O, IPAD), dtype=bf)
    wh[:, :I] = hi
    wc = np.zeros((O, ICAT), dtype=bf)
    wc[:, :I] = lo
    wc[:, I:2 * I] = hi
    return wh, wc


def build_x2(x):
    """Concat bf16 hi/lo split, t-major: [T, B, ICAT] = [Xh(700)|Xl(700)|0]."""
    bf = ml_dtypes.bfloat16
    B, T, I = x.shape
    x2 = np.zeros((T, B, ICAT), dtype=bf)
    hi = x.astype(bf)
    x2[:, :, :I] = np.swapaxes(hi, 0, 1)
    x2[:, :, I:2 * I] = np.swapaxes((x - hi.astype(np.float32)).astype(bf), 0, 1)
    return x2


def kernel(inputs: np.ndarray, W: np.ndarray, nb_steps) -> np.ndarray:
    from concourse.bass_utils import run_bass_kernel_spmd

    B, T, I = inputs.shape
    O = W.shape[0]
    assert (B, T, I, O) == (B_FULL, T_FULL, I_FULL, O_FULL), (B, T, I, O)
    assert int(nb_steps) == T

    key = (B, T, I, O)
    if key not in _CACHE:
        _CACHE[key] = build_program(B // NCORES, T, I, O)
    nc, meta = _CACHE[key]

    bf = ml_dtypes.bfloat16
    x = np.ascontiguousarray(inputs, dtype=np.float32)
    w = np.ascontiguousarray(W, dtype=np.float32)

    x2 = build_x2(x)
    wh, wc = build_w(w)
    ident = np.eye(128, dtype=bf)

    B_L = B // NCORES
    in_maps = [
        {
            "x2": np.ascontiguousarray(x2[:, c * B_L:(c + 1) * B_L]),
            "wh": wh,
            "wc": wc,
            "ident": ident,
        }
        for c in range(NCORES)
    ]
    results = run_bass_kernel_spmd(nc, in_maps, core_ids=list(range(NCORES)))
    outs = [r["out"] for r in results.results]
    return np.concatenate(outs, axis=0)
